# revision 1
# baseline (speedup 1.0000x reference)
"""CoreHybridBlock Trainium2 kernel: builder + host glue (work version).

Per-core program (one batch element per core):
  natural layout = [token(part), feature(free)], transposed = [feature(part), token(free)]

  per chunk of C tokens:
    load x,v natural -> rmsnorm(x) natural -> PE-transpose -> xnT
    uvT   = Wconv^T @ xnT   (conv in  proj, 2*dc=512 rows)
    conv  = depthwise K=3 along free dim via tensor_scalar FMA chain + silu gate
    xssmT/dtT/BmT/CmT from matmuls with xnT
    dt chain: clip/softplus/clip ; decay = exp(dt*A) via ACT Exp(scale=A)
    B/C row-norm via ones-matmul partition reduce + bcast matmul
    inp = dt*x_ssm*Bfull ; hsT = tensor_tensor_scan(decay, inp) ; state chains chunks
    yT  = hs*Cfull + D*x_ssm ; y2T = ssm_out^T @ yT
    mixer natural via operand swap (lhsT = concat acts) ; v_new = beta*v + mixer
    x2 = x + v_new ; rmsnorm(x2) -> transpose -> nT
    FFN: aT/bT = W1^T/W3^T @ nT per kf tile ; h = silu(a)*b
         ffn natural via operand swap (lhsT = h slice, rhs = w2 tile) accumulate
    x_out = x2 + ffn ; DMA v_new, x_out
"""

import ml_dtypes
import numpy as np
import bass_rust
import concourse.bass as bass
import concourse.tile as tile
from concourse import mybir
from concourse.bass_utils import run_bass_kernel_spmd

F32 = mybir.dt.float32
F32R = mybir.dt.float32r
BF16 = mybir.dt.bfloat16
AF = mybir.ActivationFunctionType
OP = mybir.AluOpType

D_MODEL, D_CONV, D_MAMBA = 512, 256, 256
DSTATE, N_HEADS, KCONV, FFN = 64, 4, 3, 2048
EPS = 1e-6


# ---------------------------------------------------------------- wait split
def split_waits(nc, max_w=1):
    """walrus in this container rejects >~1 sync wait per instruction on some
    instruction types (the Tile end-drain carries one wait per live
    semaphore).  Hoist excess waits onto same-engine NoOps placed before the
    offending instruction."""
    cnt = 0
    for f in nc.m.functions:
        for bb in f.blocks:
            new_list = []
            changed = False
            for inst in bb.instructions:
                si = inst.sync_info
                waits = list(si.on_wait) if si is not None and si.on_wait else []
                if len(waits) > max_w:
                    changed = True
                    extra = waits[max_w:]
                    si.on_wait = waits[:max_w]
                    for j in range(0, len(extra), max_w):
                        cnt += 1
                        nop = bass_rust.InstNoOp(
                            name=f"I-waitsplit-{cnt}", ins=[], outs=[]
                        )
                        nop.engine = inst.engine
                        nop.sync_info = bass_rust.SyncInfo(
                            on_wait=extra[j : j + max_w], on_update=[]
                        )
                        new_list.append(nop)
                new_list.append(inst)
            if changed:
                bb.instructions = new_list
    return cnt


# ---------------------------------------------------------------- program
def build_program(L, C, beta, split=True, fast=True):
    """One-core program; SPMD over 8 cores with different x/v slices."""
    NCH = L // C
    NSUB = C // 128  # L-subtiles per chunk (natural layout)
    nc = bass.Bass()

    # ---- dram I/O
    x_d = nc.dram_tensor("x", [L, D_MODEL], F32, kind="ExternalInput")
    v_d = nc.dram_tensor("v", [L, D_MODEL], F32, kind="ExternalInput")
    wconv_d = nc.dram_tensor("w_conv", [D_MODEL, 2 * D_CONV], F32, kind="ExternalInput")
    wxp_d = nc.dram_tensor("w_xproj", [D_MODEL, D_MAMBA], F32, kind="ExternalInput")
    wdt_d = nc.dram_tensor("w_dt", [D_MODEL, D_MAMBA], F32, kind="ExternalInput")
    wb_d = nc.dram_tensor("w_b", [D_MODEL, DSTATE], F32, kind="ExternalInput")
    wc_d = nc.dram_tensor("w_c", [D_MODEL, DSTATE], F32, kind="ExternalInput")
    wssm_d = nc.dram_tensor("w_ssmout", [D_MAMBA, D_MAMBA], F32, kind="ExternalInput")
    wop_d = nc.dram_tensor("w_outproj", [D_MODEL, D_MODEL], F32, kind="ExternalInput")
    w1_d = nc.dram_tensor("w1", [D_MODEL, FFN], BF16, kind="ExternalInput")
    w3_d = nc.dram_tensor("w3", [D_MODEL, FFN], BF16, kind="ExternalInput")
    w2_d = nc.dram_tensor("w2", [FFN, D_MODEL], BF16, kind="ExternalInput")
    avec_d = nc.dram_tensor("a_vec", [D_MAMBA, 1], F32, kind="ExternalInput")
    dtb_d = nc.dram_tensor("dtb_vec", [D_MAMBA, 1], F32, kind="ExternalInput")
    dvec_d = nc.dram_tensor("d_vec", [D_MAMBA, 1], F32, kind="ExternalInput")
    convb_d = nc.dram_tensor("convb_vec", [D_CONV, 1], F32, kind="ExternalInput")
    convw_d = nc.dram_tensor("convw", [D_CONV, KCONV], F32, kind="ExternalInput")
    mask2_d = nc.dram_tensor("mask2", [128, 2], F32, kind="ExternalInput")
    selb_d = nc.dram_tensor("selb", [2, 128], F32, kind="ExternalInput")
    selc_d = nc.dram_tensor("selc", [2, 128], F32, kind="ExternalInput")
    ident_d = nc.dram_tensor("ident", [128, 128], F32, kind="ExternalInput")

    xo_d = nc.dram_tensor("x_out", [L, D_MODEL], F32, kind="ExternalOutput")
    vo_d = nc.dram_tensor("v_out", [L, D_MODEL], F32, kind="ExternalOutput")

    with tile.TileContext(nc) as tc:
        with (
            tc.tile_pool(name="consts", bufs=1) as cp,
            tc.tile_pool(name="state", bufs=1) as sp,
            tc.tile_pool(name="innat", bufs=2 * NSUB) as pin,
            tc.tile_pool(name="norm", bufs=2) as pnorm,
            tc.tile_pool(name="xn", bufs=NSUB + 1) as pxn,
            tc.tile_pool(name="xnT", bufs=8) as pxnT,
            tc.tile_pool(name="convp", bufs=3) as pconv,
            tc.tile_pool(name="ssm", bufs=2) as pssm,
            tc.tile_pool(name="bc", bufs=2) as pbc,
            tc.tile_pool(name="nat2", bufs=2 * NSUB) as pnat2,
            tc.tile_pool(name="ffna", bufs=3) as pffna,
            tc.tile_pool(name="psC", bufs=4, space="PSUM") as psC,
            tc.tile_pool(name="psN", bufs=4, space="PSUM") as psN,
        ):
            MDT = F32R if fast else F32

            def mm(out, lhsT, rhs, start, stop):
                nc.tensor.matmul(out=out, lhsT=lhsT, rhs=rhs, start=start, stop=stop)

            # ---------------- constants / weights resident in SBUF
            def load_const(name, dram_ap, shape, dt=F32):
                t = cp.tile(shape, dt, name=name, tag=name)
                src_ap = dram_ap.bitcast(dt) if dt is F32R else dram_ap
                nc.sync.dma_start(out=t, in_=src_ap)
                return t

            wconv_sb = [
                load_const(f"wconv{k}", wconv_d[k * 128 : (k + 1) * 128, :], [128, 2 * D_CONV], MDT)
                for k in range(4)
            ]
            wxp_sb = [
                load_const(f"wxp{k}", wxp_d[k * 128 : (k + 1) * 128, :], [128, D_MAMBA], MDT)
                for k in range(4)
            ]
            wdt_sb = [
                load_const(f"wdt{k}", wdt_d[k * 128 : (k + 1) * 128, :], [128, D_MAMBA], MDT)
                for k in range(4)
            ]
            wb_sb = [
                load_const(f"wb{k}", wb_d[k * 128 : (k + 1) * 128, :], [128, DSTATE])
                for k in range(4)
            ]
            wc_sb = [
                load_const(f"wc{k}", wc_d[k * 128 : (k + 1) * 128, :], [128, DSTATE])
                for k in range(4)
            ]
            wssm_sb = [
                load_const(f"wssm{k}", wssm_d[k * 128 : (k + 1) * 128, :], [128, D_MAMBA], MDT)
                for k in range(2)
            ]
            wop_sb = [
                load_const(f"wop{k}", wop_d[k * 128 : (k + 1) * 128, :], [128, D_MODEL], MDT)
                for k in range(4)
            ]
            w2_sb = [
                load_const(f"w2_{k}", w2_d[k * 128 : (k + 1) * 128, :], [128, D_MODEL], BF16)
                for k in range(16)
            ]
            w1_sb = [
                load_const(f"w1_{k}", w1_d[k * 128 : (k + 1) * 128, :], [128, FFN], BF16)
                for k in range(4)
            ]
            w3_sb = [
                load_const(f"w3_{k}", w3_d[k * 128 : (k + 1) * 128, :], [128, FFN], BF16)
                for k in range(4)
            ]
            avec = [
                load_const(f"avec{m}", avec_d[m * 128 : (m + 1) * 128, :], [128, 1])
                for m in range(2)
            ]
            dtb = [
                load_const(f"dtb{m}", dtb_d[m * 128 : (m + 1) * 128, :], [128, 1])
                for m in range(2)
            ]
            dvec = [
                load_const(f"dvec{m}", dvec_d[m * 128 : (m + 1) * 128, :], [128, 1])
                for m in range(2)
            ]
            convb = [
                load_const(f"convb{m}", convb_d[m * 128 : (m + 1) * 128, :], [128, 1])
                for m in range(2)
            ]
            convw = [
                load_const(f"convw{m}", convw_d[m * 128 : (m + 1) * 128, :], [128, KCONV])
                for m in range(2)
            ]
            mask2 = load_const("mask2", mask2_d[:, :], [128, 2])
            selb = load_const("selb", selb_d[:, :], [2, 128])
            selc = load_const("selc", selc_d[:, :], [2, 128])
            ident = load_const("ident", ident_d[:, :], [128, 128])

            eps_sb = cp.tile([128, 1], F32, name="eps_sb", tag="eps_sb")
            nc.vector.memset(eps_sb, EPS)
            one_sb = cp.tile([128, 1], F32, name="one_sb", tag="one_sb")
            nc.vector.memset(one_sb, 1.0)

            # ---------------- persistent cross-chunk state
            h_st = [sp.tile([128, 1], F32, name=f"hst{m}", tag=f"hst{m}") for m in range(2)]
            u_halo = [sp.tile([128, 2], F32, name=f"uhalo{m}", tag=f"uhalo{m}") for m in range(2)]
            for m in range(2):
                nc.vector.memset(h_st[m], 0.0)
                nc.vector.memset(u_halo[m], 0.0)

            # ---------------- helpers
            def rmsnorm_apply(src_tiles, dst_tag):
                """fp32 rmsnorm over feature dim (natural layout); weight is
                folded into downstream matmul weights on the host."""
                out_tiles = []
                for i, xt in enumerate(src_tiles):
                    sq = pnorm.tile([128, D_MODEL], F32, name="sq", tag="sq")
                    ssq = pnorm.tile([128, 1], F32, name="ssq", tag="ssq")
                    nc.scalar.activation(out=sq, in_=xt, func=AF.Square, accum_out=ssq)
                    r = pnorm.tile([128, 1], F32, name="rr", tag="rr")
                    nc.scalar.activation(
                        out=r, in_=ssq, func=AF.Sqrt, scale=1.0 / D_MODEL, bias=eps_sb
                    )
                    nc.vector.reciprocal(out=r, in_=r)
                    xn = pxn.tile([128, D_MODEL], F32, name=dst_tag, tag=dst_tag)
                    nc.vector.tensor_scalar(
                        out=xn, in0=xt, scalar1=r, scalar2=None, op0=OP.mult
                    )
                    out_tiles.append(xn)
                return out_tiles

            def transpose_tiles(nat_tiles, dst_tag, dt):
                """[NSUB x [128, D_MODEL]] natural -> 4 x [128, C] transposed."""
                outT = []
                for d in range(4):
                    ps = psC.tile([128, C], F32, name="psC", tag="psC")
                    for i in range(NSUB):
                        nc.tensor.transpose(
                            out=ps[:, i * 128 : (i + 1) * 128],
                            in_=nat_tiles[i][:, d * 128 : (d + 1) * 128],
                            identity=ident,
                        )
                    t = pxnT.tile([128, C], dt, name=dst_tag, tag=dst_tag)
                    nc.vector.tensor_copy(out=t, in_=ps)
                    outT.append(t)
                return outT

            # ---------------- main chunk loop
            for c in range(NCH):
                row0 = c * C

                x_nat, v_nat = [], []
                for i in range(NSUB):
                    xt = pin.tile([128, D_MODEL], F32, name="xnat", tag="xnat")
                    nc.sync.dma_start(
                        out=xt, in_=x_d[row0 + i * 128 : row0 + (i + 1) * 128, :]
                    )
                    x_nat.append(xt)
                    vt = pin.tile([128, D_MODEL], F32, name="vnat", tag="vnat")
                    nc.sync.dma_start(
                        out=vt, in_=v_d[row0 + i * 128 : row0 + (i + 1) * 128, :]
                    )
                    v_nat.append(vt)

                xn_nat = rmsnorm_apply(x_nat, "xn")
                xnT = transpose_tiles(xn_nat, "xnT", MDT)

                # ---- conv input projection: uvT[m] m<4 (u: m 0-1, gate: m 2-3)
                u_ext, g_s = [], []
                for m in range(4):
                    ps = psC.tile([128, C], F32, name="psC", tag="psC")
                    for k in range(4):
                        mm(
                            out=ps,
                            lhsT=wconv_sb[k][:, m * 128 : (m + 1) * 128],
                            rhs=xnT[k],
                            start=(k == 0),
                            stop=(k == 3),
                        )
                    if m < 2:
                        ue = pconv.tile([128, C + 2], F32, name="uext", tag="uext")
                        nc.vector.tensor_copy(out=ue[:, 2 : C + 2], in_=ps)
                        nc.vector.tensor_copy(out=ue[:, 0:2], in_=u_halo[m])
                        nc.vector.tensor_copy(out=u_halo[m], in_=ue[:, C : C + 2])
                        u_ext.append(ue)
                    else:
                        gsig = pconv.tile([128, C], F32, name="gsig", tag="gsig")
                        nc.scalar.activation(out=gsig, in_=ps, func=AF.Sigmoid)
                        gs = pconv.tile([128, C], F32, name="gs", tag="gs")
                        nc.vector.tensor_mul(out=gs, in0=ps, in1=gsig)
                        g_s.append(gs)

                conv_out = []
                for m in range(2):
                    cc = pconv.tile([128, C], F32, name="cc", tag="cc")
                    nc.vector.tensor_scalar(
                        out=cc,
                        in0=u_ext[m][:, 0:C],
                        scalar1=convw[m][:, 0:1],
                        scalar2=convb[m],
                        op0=OP.mult,
                        op1=OP.add,
                    )
                    for kk in (1, 2):
                        nc.vector.scalar_tensor_tensor(
                            out=cc,
                            in0=u_ext[m][:, kk : C + kk],
                            scalar=convw[m][:, kk : kk + 1],
                            in1=cc,
                            op0=OP.mult,
                            op1=OP.add,
                        )
                    co = pconv.tile([128, C], MDT, name="convout", tag="convout")
                    nc.vector.tensor_mul(out=co, in0=cc, in1=g_s[m])
                    conv_out.append(co)

                # ---- x_ssm^T and dt^T and decay^T
                xssmT, dtT, decayT = [], [], []
                for m in range(2):
                    ps = psC.tile([128, C], F32, name="psC", tag="psC")
                    for k in range(4):
                        mm(
                            out=ps,
                            lhsT=wxp_sb[k][:, m * 128 : (m + 1) * 128],
                            rhs=xnT[k],
                            start=(k == 0),
                            stop=(k == 3),
                        )
                    xs = pssm.tile([128, C], F32, name="xssm", tag="xssm")
                    nc.vector.tensor_copy(out=xs, in_=ps)
                    xssmT.append(xs)
                for m in range(2):
                    ps = psC.tile([128, C], F32, name="psC", tag="psC")
                    for k in range(4):
                        mm(
                            out=ps,
                            lhsT=wdt_sb[k][:, m * 128 : (m + 1) * 128],
                            rhs=xnT[k],
                            start=(k == 0),
                            stop=(k == 3),
                        )
                    dt_t = pssm.tile([128, C], F32, name="dtt", tag="dtt")
                    # clip(raw + dt_b, -10, 5)
                    nc.vector.tensor_scalar(
                        out=dt_t, in0=ps, scalar1=dtb[m], scalar2=-10.0,
                        op0=OP.add, op1=OP.max,
                    )
                    nc.vector.tensor_scalar(
                        out=dt_t, in0=dt_t, scalar1=5.0, scalar2=None, op0=OP.min
                    )
                    sp_t = pssm.tile([128, C], F32, name="dtsp", tag="dtsp")
                    nc.scalar.activation(out=sp_t, in_=dt_t, func=AF.Exp)
                    nc.scalar.activation(out=sp_t, in_=sp_t, func=AF.Ln, bias=one_sb)
                    dt_f = pssm.tile([128, C], F32, name="dtf", tag="dtf")
                    nc.vector.tensor_scalar(
                        out=dt_f, in0=sp_t, scalar1=1e-4, scalar2=0.1,
                        op0=OP.max, op1=OP.min,
                    )
                    dtT.append(dt_f)
                    dec = pssm.tile([128, C], F32, name="dec", tag="dec")
                    nc.scalar.activation(out=dec, in_=dt_f, func=AF.Exp, scale=avec[m])
                    decayT.append(dec)

                # ---- B/C projections + row norm
                ps_bc = psC.tile([128, C], F32, name="psC", tag="psC")
                for k in range(4):
                    nc.tensor.matmul(
                        out=ps_bc[0:64, :], lhsT=wb_sb[k], rhs=xnT[k].bitcast(F32),
                        start=(k == 0), stop=(k == 3),
                    )
                for k in range(4):
                    nc.tensor.matmul(
                        out=ps_bc[64:128, :], lhsT=wc_sb[k], rhs=xnT[k].bitcast(F32),
                        start=(k == 0), stop=(k == 3),
                    )
                sq_bc = pbc.tile([128, C], F32, name="sqbc", tag="sqbc")
                nc.scalar.activation(out=sq_bc, in_=ps_bc, func=AF.Square)
                ps_sums = psC.tile([128, C], F32, name="psC", tag="psC")
                nc.tensor.matmul(
                    out=ps_sums[0:2, :], lhsT=mask2, rhs=sq_bc, start=True, stop=True
                )
                r_bc = pbc.tile([2, C], F32, name="rbc", tag="rbc")
                nc.scalar.activation(out=r_bc, in_=ps_sums[0:2, :], func=AF.Sqrt)
                nc.vector.reciprocal(out=r_bc, in_=r_bc)
                nc.vector.tensor_scalar(
                    out=r_bc, in0=r_bc, scalar1=1.0, scalar2=None, op0=OP.min
                )
                bm_s = pbc.tile([128, C], F32, name="bms", tag="bms")
                nc.vector.tensor_copy(out=bm_s, in_=ps_bc)
                ps_sB = psC.tile([128, C], F32, name="psC", tag="psC")
                nc.tensor.matmul(out=ps_sB, lhsT=selb, rhs=r_bc, start=True, stop=True)
                ps_sC = psC.tile([128, C], F32, name="psC", tag="psC")
                nc.tensor.matmul(out=ps_sC, lhsT=selc, rhs=r_bc, start=True, stop=True)
                b128 = pbc.tile([128, C], F32, name="b128", tag="b128")
                c128 = pbc.tile([128, C], F32, name="c128", tag="c128")
                nc.sync.dma_start(out=b128[0:64, :], in_=bm_s[0:64, :])
                nc.sync.dma_start(out=b128[64:128, :], in_=bm_s[0:64, :])
                nc.sync.dma_start(out=c128[0:64, :], in_=bm_s[64:128, :])
                nc.sync.dma_start(out=c128[64:128, :], in_=bm_s[64:128, :])
                nc.vector.tensor_mul(out=b128, in0=b128, in1=ps_sB)
                nc.vector.tensor_mul(out=c128, in0=c128, in1=ps_sC)

                # ---- scan
                yT = []
                for m in range(2):
                    inp = pssm.tile([128, C], F32, name="inp", tag="inp")
                    nc.vector.tensor_mul(out=inp, in0=dtT[m], in1=xssmT[m])
                    nc.vector.tensor_mul(out=inp, in0=inp, in1=b128)
                    hs = pssm.tile([128, C], F32, name="hs", tag="hs")
                    nc.vector.tensor_tensor_scan(
                        out=hs, data0=decayT[m], data1=inp, initial=h_st[m],
                        op0=OP.mult, op1=OP.add,
                    )
                    nc.vector.tensor_copy(out=h_st[m], in_=hs[:, C - 1 : C])
                    hc = pssm.tile([128, C], F32, name="hc", tag="hc")
                    nc.vector.tensor_mul(out=hc, in0=hs, in1=c128)
                    yt = pssm.tile([128, C], MDT, name="yt", tag="yt")
                    nc.vector.scalar_tensor_tensor(
                        out=yt, in0=xssmT[m], scalar=dvec[m], in1=hc,
                        op0=OP.mult, op1=OP.add,
                    )
                    yT.append(yt)

                # ---- ssm out proj
                y2T = []
                for m in range(2):
                    ps = psC.tile([128, C], F32, name="psC", tag="psC")
                    for k in range(2):
                        mm(
                            out=ps,
                            lhsT=wssm_sb[k][:, m * 128 : (m + 1) * 128],
                            rhs=yT[k],
                            start=(k == 0),
                            stop=(k == 1),
                        )
                    y2 = pssm.tile([128, C], MDT, name="y2", tag="y2")
                    nc.vector.tensor_copy(out=y2, in_=ps)
                    y2T.append(y2)

                # ---- mixer (natural layout via operand swap) + velocity/residual
                mix_lhsT = [conv_out[0], conv_out[1], y2T[0], y2T[1]]
                x2_nat = []
                for li in range(NSUB):
                    ps = psN.tile([128, D_MODEL], F32, name="psN", tag="psN")
                    for k in range(4):
                        mm(
                            out=ps,
                            lhsT=mix_lhsT[k][:, li * 128 : (li + 1) * 128],
                            rhs=wop_sb[k],
                            start=(k == 0),
                            stop=(k == 3),
                        )
                    vn = pnat2.tile([128, D_MODEL], F32, name="vnew", tag="vnew")
                    nc.vector.scalar_tensor_tensor(
                        out=vn, in0=v_nat[li], scalar=beta, in1=ps,
                        op0=OP.mult, op1=OP.add,
                    )
                    nc.sync.dma_start(
                        out=vo_d[row0 + li * 128 : row0 + (li + 1) * 128, :], in_=vn
                    )
                    x2 = pnat2.tile([128, D_MODEL], F32, name="x2", tag="x2")
                    nc.vector.tensor_add(out=x2, in0=x_nat[li], in1=vn)
                    x2_nat.append(x2)

                # ---- FFN
                n_nat = rmsnorm_apply(x2_nat, "n2")
                nT = transpose_tiles(n_nat, "nT", BF16)

                ps_ffn = [psN.tile([128, D_MODEL], F32, name="psN", tag="psN") for _ in range(NSUB)]
                for kf in range(16):
                    ps_a = psC.tile([128, C], F32, name="psC", tag="psC")
                    for k in range(4):
                        mm(
                            out=ps_a,
                            lhsT=w1_sb[k][:, kf * 128 : (kf + 1) * 128], rhs=nT[k],
                            start=(k == 0), stop=(k == 3),
                        )
                    ps_b = psC.tile([128, C], F32, name="psC", tag="psC")
                    for k in range(4):
                        mm(
                            out=ps_b,
                            lhsT=w3_sb[k][:, kf * 128 : (kf + 1) * 128], rhs=nT[k],
                            start=(k == 0), stop=(k == 3),
                        )
                    h_a = pffna.tile([128, C], F32, name="ha", tag="ha")
                    nc.scalar.activation(out=h_a, in_=ps_a, func=AF.Sigmoid)
                    h_t = pffna.tile([128, C], F32, name="ht", tag="ht")
                    nc.vector.tensor_mul(out=h_t, in0=ps_a, in1=h_a)
                    h_sb = pffna.tile([128, C], BF16, name="hsb", tag="hsb")
                    nc.vector.tensor_mul(out=h_sb, in0=ps_b, in1=h_t)
                    for li in range(NSUB):
                        mm(
                            out=ps_ffn[li],
                            lhsT=h_sb[:, li * 128 : (li + 1) * 128],
                            rhs=w2_sb[kf],
                            start=(kf == 0),
                            stop=(kf == 15),
                        )

                for li in range(NSUB):
                    xf = pnat2.tile([128, D_MODEL], F32, name="xfin", tag="xfin")
                    nc.vector.tensor_add(out=xf, in0=x2_nat[li], in1=ps_ffn[li])
                    nc.sync.dma_start(
                        out=xo_d[row0 + li * 128 : row0 + (li + 1) * 128, :], in_=xf
                    )

    if split:
        split_waits(nc)
    return nc


# ---------------------------------------------------------------- host glue
def prep_weights(inputs):
    """Host-side preprocessing: fold norm weights into matmul weights,
    precompute A = -exp(A_log), beta, and small constant matrices."""
    f = lambda a: np.asarray(a, dtype=np.float32)
    pre_w = f(inputs["pre_norm_w"])[:, None]
    ffn_w = f(inputs["ffn_norm_w"])[:, None]
    A = -np.exp(f(inputs["A_log"]).reshape(-1))
    mask2 = np.zeros((128, 2), np.float32)
    mask2[0:64, 0] = 1.0
    mask2[64:128, 1] = 1.0
    selb = np.zeros((2, 128), np.float32)
    selb[0, :] = 1.0
    selc = np.zeros((2, 128), np.float32)
    selc[1, :] = 1.0
    beta = float(1.0 / (1.0 + np.exp(-f(inputs["log_beta"]))))
    w = {
        "w_conv": np.ascontiguousarray(pre_w * f(inputs["conv_in_w"])),
        "w_xproj": np.ascontiguousarray(pre_w * f(inputs["x_proj_w"])),
        "w_dt": np.ascontiguousarray(pre_w * f(inputs["dt_w"])),
        "w_b": np.ascontiguousarray(pre_w * f(inputs["B_w"])),
        "w_c": np.ascontiguousarray(pre_w * f(inputs["C_w"])),
        "w_ssmout": np.ascontiguousarray(f(inputs["ssm_out_w"])),
        "w_outproj": np.ascontiguousarray(f(inputs["out_proj_w"])),
        "w1": np.ascontiguousarray((ffn_w * f(inputs["w1"])).astype(ml_dtypes.bfloat16)),
        "w3": np.ascontiguousarray((ffn_w * f(inputs["w3"])).astype(ml_dtypes.bfloat16)),
        "w2": np.ascontiguousarray(f(inputs["w2"]).astype(ml_dtypes.bfloat16)),
        "a_vec": A[:, None].copy(),
        "dtb_vec": f(inputs["dt_b"])[:, None].copy(),
        "d_vec": f(inputs["D"])[:, None].copy(),
        "convb_vec": f(inputs["conv_dw_b"])[:, None].copy(),
        "convw": np.ascontiguousarray(f(inputs["conv_dw_w"])),
        "mask2": mask2,
        "selb": selb,
        "selc": selc,
        "ident": np.eye(128, dtype=np.float32),
    }
    return w, beta


def run(inputs, L=4096, C=256, trace=False):
    w, beta = prep_weights(inputs)
    nc = build_program(L, C, beta)
    x = np.asarray(inputs["x"], np.float32)
    v = np.asarray(inputs["velocity"], np.float32)
    n_cores = x.shape[0]
    in_maps = []
    for b in range(n_cores):
        m = dict(w)
        m["x"] = np.ascontiguousarray(x[b])
        m["v"] = np.ascontiguousarray(v[b])
        in_maps.append(m)
    res = run_bass_kernel_spmd(nc, in_maps, core_ids=list(range(n_cores)), trace=trace)
    x_out = np.stack([res.results[b]["x_out"] for b in range(n_cores)])
    v_out = np.stack([res.results[b]["v_out"] for b in range(n_cores)])
    return (x_out, v_out), res



CHUNK = 256

_PROG_CACHE = {}


def kernel(**inputs):
    """Full-input entry point: shard batch over the 8 NeuronCores (one batch
    element per core; the scan state is per-(batch,channel) so this is
    embarrassingly parallel), run the Bass program SPMD, regather."""
    w, beta = prep_weights(inputs)
    x = np.asarray(inputs["x"], np.float32)
    v = np.asarray(inputs["velocity"], np.float32)
    n_cores, L, _ = x.shape
    key = (L, CHUNK, beta)
    if key not in _PROG_CACHE:
        _PROG_CACHE[key] = build_program(L, CHUNK, beta)
    nc = _PROG_CACHE[key]
    in_maps = []
    for b in range(n_cores):
        m = dict(w)
        m["x"] = np.ascontiguousarray(x[b])
        m["v"] = np.ascontiguousarray(v[b])
        in_maps.append(m)
    res = run_bass_kernel_spmd(nc, in_maps, core_ids=list(range(n_cores)))
    x_out = np.stack([res.results[b]["x_out"] for b in range(n_cores)])
    v_out = np.stack([res.results[b]["v_out"] for b in range(n_cores)])
    return (x_out, v_out)

